# revision 1
# baseline (speedup 1.0000x reference)
"""Batched 20x20 SPD covariance-matrix inversion on 8 Trainium2 NeuronCores.

For each of 131072 batches: build C = exp(-1.5 * pairwise_dist(pos)) + 0.01*I
from 20 2-D points, return C^{-1}.

Strategy (per core, data-parallel over batch):
 - batch-major layout: each of 128 SBUF partitions holds M matrices' full
   20x20 (400 fp32) in the free dim; CHUNKS chunks of M per partition.
 - symmetric sweep operator (Gauss-Jordan preserving symmetry): only the
   upper triangle is updated each pivot, covered by 4 rectangles so the
   DVE ops stay large; final negate + mirror to emit the full inverse.
 - ACT (scalar engine) does square/sqrt/exp and the pivot-column gathers;
   DVE does the rank-1 updates via stride-0 broadcast access patterns.
"""

import numpy as np

import concourse.bass as bass  # noqa: F401  (registers engine APIs)
import concourse.tile as tile
from concourse import bacc, mybir
from concourse.bass_utils import run_bass_kernel_spmd

N = 20                  # matrix dim
D = 2                   # coord dim
PHI = 1.5
TAU = 0.01
P = 128                 # SBUF partitions
N_CORES = 8
B_TOTAL = 131072
B_CORE = B_TOTAL // N_CORES   # 16384

F32 = mybir.dt.float32
AF = mybir.ActivationFunctionType
OP = mybir.AluOpType

# Upper-triangle rectangle cover: rows [r0,r1) x cols [r0,N)
RECTS = [(0, 5), (5, 10), (10, 15), (15, 20)]
# rows [0, GP_SPLIT) of each rect go to GPSIMD, rest to DVE (0 = all DVE)
GP_SPLIT = 0
GATHER_SCALAR = True   # pivot-column gather on ACT (else DVE)
MIRROR_GP = False      # alternate mirror copies DVE/GPSIMD


def emit_kernel(tc, pos_ap, out_ap, b_core, m_chunk):
    """Emit the per-core program. pos: [b_core, 40] f32, out: [b_core, 400] f32."""
    nc = tc.nc
    chunks = b_core // (P * m_chunk)
    assert b_core == P * m_chunk * chunks
    M = m_chunk
    H = N // 2  # cov build column-half width

    pos_r = pos_ap.rearrange("(p c m) f -> p c (m f)", p=P, c=chunks)
    out_r = out_ap.rearrange("(p c m) f -> p c (m f)", p=P, c=chunks)

    with (
        tc.tile_pool(name="pos", bufs=2) as pos_pool,
        tc.tile_pool(name="A", bufs=2) as a_pool,
        tc.tile_pool(name="cov", bufs=1) as cov_pool,
        tc.tile_pool(name="rect", bufs=2) as rect_pool,
        tc.tile_pool(name="small", bufs=2) as small_pool,
    ):
        for c in range(chunks):
            pos_t = pos_pool.tile([P, M * N * D], F32)
            nc.sync.dma_start(pos_t[:, :], pos_r[:, c, :])
            posv = pos_t[:, :].rearrange("p (m i d) -> p m i d", m=M, i=N)

            A = a_pool.tile([P, M * N * N], F32)
            A4 = A[:, :].rearrange("p (m i j) -> p m i j", m=M, i=N)

            # ---- covariance build: A = exp(-PHI * dist) (+ TAU on diag) ----
            for h in range(2):
                jsl = slice(h * H, (h + 1) * H)
                reg = A4[:, :, :, jsl]
                xi = posv[:, :, :, 0].unsqueeze(3).broadcast_to([P, M, N, H])
                xj = posv[:, :, jsl, 0].unsqueeze(2).broadcast_to([P, M, N, H])
                nc.vector.tensor_sub(reg, xi, xj)
                nc.scalar.square(reg, reg)
                dy = cov_pool.tile([P, M * N * H], F32)
                dyv = dy[:, :].rearrange("p (m i j) -> p m i j", m=M, i=N)
                yi = posv[:, :, :, 1].unsqueeze(3).broadcast_to([P, M, N, H])
                yj = posv[:, :, jsl, 1].unsqueeze(2).broadcast_to([P, M, N, H])
                nc.vector.tensor_sub(dyv, yi, yj)
                nc.scalar.square(dyv, dyv)
                nc.vector.tensor_add(reg, reg, dyv)
                nc.scalar.sqrt(reg, reg)
                nc.scalar.activation(reg, reg, AF.Exp, scale=-PHI)

            Av = A[:, :].rearrange("p (m x) -> p m x", m=M)
            diag = Av[:, :, 0 : N * N : N + 1]
            nc.vector.tensor_scalar_add(diag, diag, TAU)

            # ---- sweep all 20 pivots ----
            for k in range(N):
                cK = small_pool.tile([P, M * N], F32, tag="c")
                crK = small_pool.tile([P, M * N], F32, tag="cr")
                rK = small_pool.tile([P, M], F32, tag="r")
                c3 = cK[:, :].rearrange("p (m i) -> p m i", m=M)
                cr3 = crK[:, :].rearrange("p (m i) -> p m i", m=M)

                # gather pivot column from upper storage (ACT engine)
                gat = nc.scalar.copy if GATHER_SCALAR else nc.vector.tensor_copy
                if k:
                    gat(c3[:, :, :k], A4[:, :, :k, k])
                gat(c3[:, :, k:], A4[:, :, k, k:])
                nc.vector.reciprocal(rK[:, :], c3[:, :, k])
                # diag <- -r
                nc.vector.tensor_scalar_mul(A4[:, :, k, k], rK[:, :], -1.0)
                nc.vector.memset(c3[:, :, k], 0.0)
                rb = rK[:, :].unsqueeze(2).broadcast_to([P, M, N])
                nc.vector.tensor_mul(cr3, c3, rb)
                # pivot row/col (upper parts) <- cr
                if k:
                    nc.vector.tensor_copy(A4[:, :, :k, k], cr3[:, :, :k])
                if k < N - 1:
                    nc.vector.tensor_copy(A4[:, :, k, k + 1 :], cr3[:, :, k + 1 :])
                # rank-1 update of the upper triangle (rect cover)
                for (r0, r1) in RECTS:
                    ncl = N - r0
                    for eng, a, b in (
                        (nc.gpsimd, r0, min(r1, r0 + GP_SPLIT)),
                        (nc.vector, min(r1, r0 + GP_SPLIT), r1),
                    ):
                        nr = b - a
                        if nr <= 0:
                            continue
                        tmp = rect_pool.tile([P, M * nr * ncl], F32, tag="rect")
                        tv = tmp[:, :].rearrange(
                            "p (m i j) -> p m i j", m=M, i=nr
                        )
                        cb = (
                            c3[:, :, a:b]
                            .unsqueeze(3)
                            .broadcast_to([P, M, nr, ncl])
                        )
                        crb = (
                            cr3[:, :, r0:]
                            .unsqueeze(2)
                            .broadcast_to([P, M, nr, ncl])
                        )
                        eng.tensor_mul(tv, cb, crb)
                        reg = A4[:, :, a:b, r0:]
                        eng.tensor_sub(reg, reg, tv)

            # ---- negate (full, contiguous => 2x mode) + mirror lower ----
            nc.vector.tensor_scalar_mul(A[:, :], A[:, :], -1.0)
            for i in range(N - 1):
                eng = nc.gpsimd if (MIRROR_GP and i % 2 == 0) else nc.vector
                eng.tensor_copy(A4[:, :, i + 1 :, i], A4[:, :, i, i + 1 :])

            nc.sync.dma_start(out_r[:, c, :], A[:, :])


_CACHE = {}


def build_nc(b_core=B_CORE, m_chunk=32, num_devices=N_CORES):
    key = (b_core, m_chunk, num_devices)
    if key in _CACHE:
        return _CACHE[key]
    nc = bacc.Bacc(
        "TRN2", target_bir_lowering=False, debug=False, num_devices=num_devices
    )
    pos_d = nc.dram_tensor("pos", [b_core, N * D], F32, kind="ExternalInput")
    out_d = nc.dram_tensor("out", [b_core, N * N], F32, kind="ExternalOutput")
    with tile.TileContext(nc) as tc:
        emit_kernel(tc, pos_d.ap(), out_d.ap(), b_core, m_chunk)
    nc.compile()
    _CACHE[key] = nc
    return nc


def run(pos_full, b_core=B_CORE, m_chunk=32, n_cores=N_CORES, **kw):
    """pos_full: [n_cores*b_core, 20, 2] f32 -> [n_cores*b_core, 20, 20] f32."""
    nc = build_nc(b_core, m_chunk, n_cores)
    flat = np.ascontiguousarray(
        np.asarray(pos_full, dtype=np.float32).reshape(-1, N * D)
    )
    in_maps = [
        {"pos": flat[i * b_core : (i + 1) * b_core]} for i in range(n_cores)
    ]
    res = run_bass_kernel_spmd(nc, in_maps, core_ids=list(range(n_cores)), **kw)
    out = np.concatenate([r["out"] for r in res.results], axis=0)
    return out.reshape(-1, N, N), res


def kernel(neighbor_positions, edge_list=None):
    out, _ = run(neighbor_positions)
    return out

